# revision 28
# baseline (speedup 1.0000x reference)
"""Trainium2 Bass kernel for per-token outer-product softmax attention.

Reference computation (per token t of 1600, H=256):
    k = tanh(x W0 + b0);  q = tanh(x W1 + b1)
    scores[i,j] = k[i]*q[j];  attn = softmax_j(scores);  out = attn @ x

Key algebra: k,q are tanh outputs so k[i]*q[j] in (-1,1). On [-1,1],
exp(s) is approximated by a degree-3 polynomial P(s) = sum_d c_d s^d
(coefficients least-squares tuned on the actual k*q product
distribution), and P(k_i q_j) = sum_d c_d k_i^d q_j^d is SEPARABLE.
Softmax numerator/denominator become per-token moments:
    num_i = sum_d (c_d sum_j q_j^d x_j) k_i^d
    den_i = sum_d (c_d sum_j q_j^d)     k_i^d
so the 256x256 scores tensor is never materialized. End-to-end rel_l2
vs the exact-softmax reference is ~2e-3 (gate: 2e-2).

Implementation highlights (per 128-token tile):
- The two 256x256 Dense matmuls run in bf16 (4x PE rate vs fp32) with
  fp32 PSUM accumulation, with W1|W0 concatenated into one [128,512]
  rhs so each token block needs only 2 matmul instructions.
- Numerator moment coefficients c_d are folded into the moment STT ops
  as immediate scalars; denominator moments ride the accum_out of the
  q-power STT ops themselves, so the whole moment set is 5 STTs +
  2 ACT ops.
- Both chains are evaluated in even/odd form: P(k) = u + k*v with
  u = a0 + a2 k^2, v = a1 + a3 k^2. u,v are single ACT ops reading k^2
  (available right after tanh-k, ahead of the moments), and the tail is
  one STT + one TT per chain, divide via a single-op ~51-ULP
  reciprocal.
- Input/weight/output DMAs are spread across four queues (sync/vector/
  gpsimd/scalar) so no transfer serializes behind another.

Sharding: pure data parallel over tokens, 200 tokens/core x 8 cores;
weights replicated. x^T is pre-transposed and pre-cast to bf16 on host
(layout/dtype prep only).
"""

import numpy as np
from contextlib import ExitStack

import concourse.bass as bass
import concourse.bacc as bacc
import concourse.tile as tile
from concourse import mybir
from concourse.bass_utils import run_bass_kernel_spmd

F32 = mybir.dt.float32
BF16 = mybir.dt.bfloat16
AF = mybir.ActivationFunctionType
OP = mybir.AluOpType

B, S, M, H = 4, 10, 40, 256
T = B * S * M            # 1600 tokens
NCORES = 8
TC = T // NCORES         # 200 tokens per core
BLOCKS = [(0, 128), (128, TC - 128)]

# Degree-3 exp approx on [-1,1], least-squares tuned on the actual
# k*q product distribution of this problem (end-to-end rel_l2 ~2e-3).
CEXP = [0.99857752, 0.99883974, 0.52666594, 0.17410473]

# Engine assignment knobs (tune against TimelineSim / HW):
CFG = {
    "s1_eng": "vector",    # NB: STT with accum_out is NOT legal on gpsimd
    "n2_eng": "vector",
    "n3_eng": "vector",
    "qp_eng": "vector",    # QP2/QP3 power STTs (carry den accums)
    "k2_eng": "gpsimd",
    "kvn_eng": "vector",   # num tail: kv = k*v, then sum = kv + u
    "nsum_eng": "vector",
    "kvd_eng": "gpsimd",   # den tail
    "dsum_eng": "gpsimd",
    "a2d_eng": "vector",
    "omul_eng": "vector",
    "j0_eng": "scalar",    # num m0 accum: scalar(ACT) | vector(DVE TS)
    "pairs_eng": "scalar",  # u/v even-odd pairs: scalar | vector
    "recip": "fast",       # fast (1 DVE op) | approx (2) | exact
    "mm_split": True,      # separate Q/K matmul groups so tanh-Q starts early
    "num_style": "stt",    # stt (AP-scalar Horner, DVE) | pairs (even/odd)
    "den_style": "pairs",  # ditto for the denominator
    "k3_eng": "gpsimd",
    "mom_style": "stt",    # stt (proven on HW) | ttr (breaks neuronxcc here)
    "order": [             # phase-interleaved emission across the 2 blocks
        ("mm", 0), ("mm", 1),
        ("head", 0), ("head", 1),
        ("momA", 0), ("momB", 0), ("momA", 1), ("momB", 1),
        ("paird", 0), ("pairn", 0), ("taild", 0), ("tailn", 0),
        ("paird", 1), ("pairn", 1), ("taild", 1), ("tailn", 1),
        ("fin", 0), ("fin", 1),
    ],
    "scrp_bufs": 2,
    "xt_q": "gpsimd",      # DMA queue for x^T
    "w_q": "sync",         # DMA queue for weights chunk0
    "w2_q": "scalar",      # DMA queue for weights chunk1 (when split)
    "w_split": True,
    "x_q": "sync",         # DMA queue for X blocks
    "out_q": ["sync", "scalar"],  # per-block output DMA queues
}


def build_kernel(
    reps: int = 1, with_bias: bool = True, unroll: bool = False
) -> bass.Bass:
    c0, c1, c2, c3 = (float(c) for c in CEXP)
    NW = 3 * 513 if with_bias else 2 * 513
    nc = bacc.Bacc("TRN2", target_bir_lowering=False, debug=False)
    xt16 = nc.declare_dram_parameter("xt16", [128, 2, TC], BF16, isOutput=False)
    w16 = nc.declare_dram_parameter("w16", [128, NW], BF16, isOutput=False)
    xs = nc.declare_dram_parameter("xs", [TC, H], F32, isOutput=False)
    cf = nc.declare_dram_parameter("cf", [128, 4], F32, isOutput=False)
    out = nc.declare_dram_parameter("out", [TC, H], F32, isOutput=True)

    with tile.TileContext(nc) as tc, ExitStack() as ctx:
        consts = ctx.enter_context(tc.tile_pool(name="consts", bufs=1))
        io = ctx.enter_context(tc.tile_pool(name="io", bufs=2))
        work = ctx.enter_context(tc.tile_pool(name="work", bufs=2))
        scrp = ctx.enter_context(
            tc.tile_pool(name="scrp", bufs=CFG["scrp_bufs"])
        )
        mom = ctx.enter_context(tc.tile_pool(name="mom", bufs=2))
        ps = ctx.enter_context(tc.tile_pool(name="ps", bufs=2, space="PSUM"))

        E = {"vector": nc.vector, "gpsimd": nc.gpsimd, "scalar": nc.scalar,
             "sync": nc.sync}

        # Weights + x^T gate the matmuls; spread them across the three DMA
        # queues (sync HWDGE, Activation HWDGE, gpsimd SWDGE) so nothing
        # serializes: w chunk0 on sync, w chunk1 (+bias) on scalar, x^T on
        # gpsimd, X blocks behind w chunk0 on sync (needed ~3us later).
        wt = consts.tile([128, NW], BF16)
        if CFG["w_split"]:
            E[CFG["w_q"]].dma_start(out=wt[:, 0:513], in_=w16[:, 0:513])
            E[CFG["w2_q"]].dma_start(out=wt[:, 513:NW], in_=w16[:, 513:NW])
        else:
            E[CFG["w_q"]].dma_start(out=wt, in_=w16[:, :])
        xtt = consts.tile([128, 2, TC], BF16)
        E[CFG["xt_q"]].dma_start(out=xtt, in_=xt16[:, :, :])
        Xs = []
        for t0, tl in BLOCKS:
            X = consts.tile([128, H], F32, tag=f"X{t0}")
            E[CFG["x_q"]].dma_start(out=X[:tl, :], in_=xs[t0 : t0 + tl, :])
            Xs.append(X)
        cft = consts.tile([128, 4], F32)
        nc.gpsimd.dma_start(out=cft, in_=cf[:, :])
        if with_bias:
            ones1 = consts.tile([1, 128], BF16)
            nc.gpsimd.memset(ones1, 1.0)

        def body():
            # Per-block state for phase-interleaved emission: per-engine
            # queues execute in program order, so phases of the two token
            # blocks are interleaved per CFG["order"] to keep every engine
            # fed in dependency-ready order.
            st = [dict() for _ in BLOCKS]

            def ph_mm(bi):
                # Q columns first (they gate the whole moment pipeline), as
                # a separate accumulation group/bank from K's. The Q rhs has
                # a 257th all-ones column so psQ[:, 256] = sum_j x_j, giving
                # the num m0 moment for free on the idle PE.
                t0, tl = BLOCKS[bi]
                psQ = ps.tile([128, 257], F32, tag="psQ")
                psK = ps.tile([128, 256], F32, tag="psK")
                for pst, lo, hi in ((psQ, 0, 257), (psK, 257, 513)):
                    if with_bias:
                        nc.tensor.matmul(
                            pst[:tl, :], ones1[:, :tl],
                            wt[0:1, 1026 + lo : 1026 + hi],
                            start=True, stop=False,
                        )
                    nc.tensor.matmul(
                        pst[:tl, :],
                        xtt[:, 0, t0 : t0 + tl],
                        wt[:, lo:hi],
                        start=not with_bias, stop=False,
                    )
                    nc.tensor.matmul(
                        pst[:tl, :],
                        xtt[:, 1, t0 : t0 + tl],
                        wt[:, 513 + lo : 513 + hi],
                        start=False, stop=True,
                    )
                st[bi]["psQ"] = psQ[:, :]
                st[bi]["psK"] = psK[:, :]

            def ph_head(bi):
                t0, tl = BLOCKS[bi]
                X = Xs[bi][:tl, :]
                # Smom cols: 0..3 num A_d (c_d folded), 4 = c1*m1,
                # 5 = raw den m1, 6 = c2*m2, 7 = c3*m3.
                Smom = mom.tile([128, 8], F32, tag="Smom")
                Qt = work.tile([128, H], F32, tag="Qt")
                nc.scalar.activation(
                    Qt[:tl, :], st[bi]["psQ"][:tl, 0:256], AF.Tanh,
                    accum_out=Smom[:tl, 5:6],
                )
                # num m0 = c0 * sum_j x_j from the ones-column of the Q mm
                nc.scalar.activation(
                    Smom[:tl, 0:1], st[bi]["psQ"][:tl, 256:257],
                    AF.Identity, scale=c0,
                )
                Kt = work.tile([128, H], F32, tag="Kt")
                nc.scalar.activation(Kt[:tl, :], st[bi]["psK"][:tl, :], AF.Tanh)
                K2 = work.tile([128, H], F32, tag="K2")
                E[CFG["k2_eng"]].tensor_mul(K2[:tl, :], Kt[:tl, :], Kt[:tl, :])
                if "stt" in (CFG["num_style"], CFG["den_style"]):
                    K3 = work.tile([128, H], F32, tag="K3")
                    E[CFG["k3_eng"]].tensor_mul(
                        K3[:tl, :], K2[:tl, :], Kt[:tl, :]
                    )
                    st[bi]["K3"] = K3
                # c1*m1 for the v_d bias slot (tiny [tl,1] ACT op)
                nc.scalar.activation(
                    Smom[:tl, 4:5], Smom[:tl, 5:6], AF.Identity, scale=c1,
                )
                st[bi].update(Smom=Smom, Q=Qt[:tl, :], K=Kt[:tl, :], K2=K2)

            # moments via tensor_tensor_reduce with PRE-SCALED q-powers:
            # QP2' = c2 q^2 (accum -> c2 m2), QP3' = c3 q^3 (accum -> c3 m3),
            # so the den pair scales come straight from the QP accums and
            # the num moments n2/n3 need no further scaling. Fallback
            # mom_style="stt" uses scalar_tensor_tensor with raw powers and
            # per-element immediate coefficient folding (scaled powers via
            # the STT scalar slot).
            def _ttr(out_ap, in0, in1, scale, acc):
                if CFG["mom_style"] == "ttr":
                    nc.vector.tensor_tensor_reduce(
                        out=out_ap, in0=in0, in1=in1, scale=scale,
                        scalar=0.0, op0=OP.mult, op1=OP.add, accum_out=acc,
                    )
                else:
                    nc.vector.scalar_tensor_tensor(
                        out=out_ap, in0=in0, scalar=scale, in1=in1,
                        op0=OP.mult, op1=OP.mult, accum_out=acc,
                    )

            def ph_momA(bi):
                t0, tl = BLOCKS[bi]
                X = Xs[bi][:tl, :]
                Q, Smom = st[bi]["Q"], st[bi]["Smom"]
                QP2 = work.tile([128, H], F32, tag="QP2")
                _ttr(QP2[:tl, :], Q, Q, c2, Smom[:tl, 6:7])
                QP3 = work.tile([128, H], F32, tag="QP3")
                _ttr(QP3[:tl, :], QP2[:tl, :], Q, c3 / c2, Smom[:tl, 7:8])
                n3 = scrp.tile([128, H], F32, tag="n3")
                _ttr(n3[:tl, :], QP3[:tl, :], X, 1.0, Smom[:tl, 3:4])
                st[bi].update(QP2=QP2)

            def ph_momB(bi):
                t0, tl = BLOCKS[bi]
                X = Xs[bi][:tl, :]
                Q, Smom, QP2 = st[bi]["Q"], st[bi]["Smom"], st[bi]["QP2"]
                s1 = scrp.tile([128, H], F32, tag="s1")
                _ttr(s1[:tl, :], Q, X, c1, Smom[:tl, 1:2])
                n2 = scrp.tile([128, H], F32, tag="n2")
                _ttr(n2[:tl, :], QP2[:tl, :], X, 1.0, Smom[:tl, 2:3])

            # even/odd pairs: u = a0 + a2 k^2, v = a1 + a3 k^2
            def _pair(bi, tag, sc, bi_):
                t0, tl = BLOCKS[bi]
                K2 = st[bi]["K2"]
                p = scrp.tile([128, H], F32, tag=tag)
                if CFG["pairs_eng"] == "scalar":
                    nc.scalar.activation(
                        p[:tl, :], K2[:tl, :], AF.Identity,
                        scale=sc, bias=bi_,
                    )
                else:
                    nc.vector.tensor_scalar(
                        out=p[:tl, :], in0=K2[:tl, :], scalar1=sc,
                        scalar2=bi_, op0=OP.mult, op1=OP.add,
                    )
                return p

            def ph_paird(bi):
                if CFG["den_style"] == "stt":
                    return
                tl = BLOCKS[bi][1]
                Smom = st[bi]["Smom"]
                st[bi]["u_d"] = _pair(
                    bi, "u_d", Smom[:tl, 6:7], cft[:tl, 0:1]
                )
                st[bi]["v_d"] = _pair(
                    bi, "v_d", Smom[:tl, 7:8], Smom[:tl, 4:5]
                )

            def ph_pairn(bi):
                if CFG["num_style"] == "stt":
                    return
                tl = BLOCKS[bi][1]
                Smom = st[bi]["Smom"]
                st[bi]["v_n"] = _pair(
                    bi, "v_n", Smom[:tl, 3:4], Smom[:tl, 1:2]
                )
                st[bi]["u_n"] = _pair(
                    bi, "u_n", Smom[:tl, 2:3], Smom[:tl, 0:1]
                )

            # chain tails: P = u + k*v
            def ph_taild(bi):
                tl = BLOCKS[bi][1]
                K = st[bi]["K"]
                if CFG["den_style"] == "stt":
                    Smom, K2, K3 = st[bi]["Smom"], st[bi]["K2"], st[bi]["K3"]
                    t1 = scrp.tile([128, H], F32, tag="t1d")
                    nc.scalar.activation(
                        t1[:tl, :], K, AF.Identity,
                        scale=Smom[:tl, 4:5], bias=cft[:tl, 0:1],
                    )
                    u2 = scrp.tile([128, H], F32, tag="u2d")
                    nc.vector.scalar_tensor_tensor(
                        out=u2[:tl, :], in0=K2[:tl, :],
                        scalar=Smom[:tl, 6:7], in1=t1[:tl, :],
                        op0=OP.mult, op1=OP.add,
                    )
                    dsum = scrp.tile([128, H], F32, tag="dsum")
                    nc.vector.scalar_tensor_tensor(
                        out=dsum[:tl, :], in0=K3[:tl, :],
                        scalar=Smom[:tl, 7:8], in1=u2[:tl, :],
                        op0=OP.mult, op1=OP.add,
                    )
                else:
                    kvd = scrp.tile([128, H], F32, tag="kvd")
                    E[CFG["kvd_eng"]].tensor_mul(
                        kvd[:tl, :], st[bi]["v_d"][:tl, :], K
                    )
                    dsum = scrp.tile([128, H], F32, tag="dsum")
                    E[CFG["dsum_eng"]].tensor_add(
                        dsum[:tl, :], kvd[:tl, :], st[bi]["u_d"][:tl, :]
                    )
                rD = scrp.tile([128, H], F32, tag="rD")
                if CFG["recip"] == "fast":
                    nc.vector.reciprocal_approx_fast(rD[:tl, :], dsum[:tl, :])
                elif CFG["recip"] == "approx":
                    rs = scrp.tile([128, H], F32, tag="rs")
                    nc.vector.reciprocal_approx_accurate(
                        rD[:tl, :], dsum[:tl, :], rs[:tl, :]
                    )
                else:
                    nc.vector.reciprocal(rD[:tl, :], dsum[:tl, :])
                st[bi]["rD"] = rD

            def ph_tailn(bi):
                tl = BLOCKS[bi][1]
                K = st[bi]["K"]
                if CFG["num_style"] == "stt":
                    # Horner in AP-scalar STT form, no cross-engine hops
                    # after the last moment: t1 = n1*k + n0 (ACT, early);
                    # u2 = n2*k^2 + t1; num = n3*k^3 + u2.
                    Smom, K2, K3 = st[bi]["Smom"], st[bi]["K2"], st[bi]["K3"]
                    t1 = scrp.tile([128, H], F32, tag="t1n")
                    nc.scalar.activation(
                        t1[:tl, :], K, AF.Identity,
                        scale=Smom[:tl, 1:2], bias=Smom[:tl, 0:1],
                    )
                    u2 = scrp.tile([128, H], F32, tag="u2n")
                    nc.vector.scalar_tensor_tensor(
                        out=u2[:tl, :], in0=K2[:tl, :],
                        scalar=Smom[:tl, 2:3], in1=t1[:tl, :],
                        op0=OP.mult, op1=OP.add,
                    )
                    nsum = scrp.tile([128, H], F32, tag="nsum")
                    nc.vector.scalar_tensor_tensor(
                        out=nsum[:tl, :], in0=K3[:tl, :],
                        scalar=Smom[:tl, 3:4], in1=u2[:tl, :],
                        op0=OP.mult, op1=OP.add,
                    )
                    st[bi]["nsum"] = nsum
                    return
                kvn = scrp.tile([128, H], F32, tag="kvn")
                E[CFG["kvn_eng"]].tensor_mul(
                    kvn[:tl, :], st[bi]["v_n"][:tl, :], K
                )
                nsum = scrp.tile([128, H], F32, tag="nsum")
                E[CFG["nsum_eng"]].tensor_add(
                    nsum[:tl, :], kvn[:tl, :], st[bi]["u_n"][:tl, :]
                )
                st[bi]["nsum"] = nsum

            def ph_fin(bi):
                t0, tl = BLOCKS[bi]
                O = io.tile([128, H], F32, tag="O")
                E[CFG["omul_eng"]].tensor_mul(
                    O[:tl, :], st[bi]["nsum"][:tl, :], st[bi]["rD"][:tl, :]
                )
                oq = CFG["out_q"][bi % len(CFG["out_q"])]
                E[oq].dma_start(out=out[t0 : t0 + tl, :], in_=O[:tl, :])

            PH = {
                "mm": ph_mm, "head": ph_head, "momA": ph_momA,
                "momB": ph_momB, "paird": ph_paird, "pairn": ph_pairn,
                "taild": ph_taild, "tailn": ph_tailn, "fin": ph_fin,
            }
            for name, bi in CFG["order"]:
                PH[name](bi)

        if reps == 1:
            body()
        elif unroll:
            for _ in range(reps):
                body()
        else:
            with tc.For_i(0, reps, 1):
                body()

    nc.compile()
    return nc


_NCS = {}


def _get_nc(with_bias: bool = True):
    if with_bias not in _NCS:
        _NCS[with_bias] = build_kernel(with_bias=with_bias)
    return _NCS[with_bias]


def _make_in_maps(x, W0, b0, W1, b1):
    import ml_dtypes

    BF = ml_dtypes.bfloat16
    with_bias = bool(
        np.any(np.asarray(b0, np.float32)) or np.any(np.asarray(b1, np.float32))
    )
    xf = np.ascontiguousarray(np.asarray(x, np.float32).reshape(T, H))
    W0_ = np.asarray(W0, np.float32)
    W1_ = np.asarray(W1, np.float32)
    # chunk layout (stride 513): [W1_c | ones | W0_c]; the ones column
    # makes the Q matmul also produce sum_j x_j. Optional bias block at
    # 1026: [b1 | 0 | b0].
    NW = 3 * 513 if with_bias else 2 * 513
    w = np.zeros((128, NW), BF)
    for c in range(2):
        s = c * 513
        w[:, s : s + 256] = W1_[c * 128 : (c + 1) * 128, :]
        w[:, s + 256] = 1.0
        w[:, s + 257 : s + 513] = W0_[c * 128 : (c + 1) * 128, :]
    if with_bias:
        w[0, 1026:1282] = np.asarray(b1, np.float32)
        w[0, 1283:1539] = np.asarray(b0, np.float32)
    # cf col 0 carries the constant den m0 term H*c0 (used as the u_d
    # bias AP); cols 1..3 are c_1..c_3 for the den coefficient scale.
    cfarr = np.tile(np.array(CEXP, np.float32).reshape(1, 4), (128, 1))
    cfarr[:, 0] = H * np.float32(CEXP[0])
    maps = []
    for c in range(NCORES):
        sh = np.ascontiguousarray(xf[c * TC : (c + 1) * TC])  # [TC, H]
        # xt16[h, chunk, t] = sh[t, chunk*128 + h], cast to bf16
        xst = np.ascontiguousarray(
            np.transpose(sh.reshape(TC, 2, 128), (2, 1, 0))
        ).astype(BF)
        maps.append({"xt16": xst, "w16": w, "xs": sh, "cf": cfarr})
    return maps


def _ensure_axon():
    # The PJRT path needs the axon devices as jax's default platform; if a
    # caller pinned cpu before importing us, try to restore axon.
    try:
        import jax
        if not any(d.platform == "axon" for d in jax.devices()):
            jax.config.update("jax_platforms", "axon,cpu")
    except Exception:
        pass


def _run(x, W0, b0, W1, b1, trace=False, **kw):
    _ensure_axon()
    with_bias = bool(
        np.any(np.asarray(b0, np.float32)) or np.any(np.asarray(b1, np.float32))
    )
    res = run_bass_kernel_spmd(
        _get_nc(with_bias), _make_in_maps(x, W0, b0, W1, b1),
        list(range(NCORES)), trace=trace, **kw,
    )
    outs = [res.results[c]["out"] for c in range(NCORES)]
    full = np.concatenate(outs, axis=0).reshape(B, S, M, H).astype(np.float32)
    return full, res


def kernel(x, W0, b0, W1, b1):
    full, _ = _run(x, W0, b0, W1, b1, trace=False)
    return full
